# revision 17
# baseline (speedup 1.0000x reference)
"""Trainium2 Bass kernel for nn_Attention_49323404427915.

GQA attention block (B=2, T=2048, D=2048, 16 q-heads, 4 kv-heads, hd=128)
with per-head QK RMSNorm + RoPE + causal SDPA + out-projection.

Sharding over 8 cores: core c handles batch (c % 2) and q-head group
(c // 2) of 4 consecutive q-heads sharing one kv head (tensor-parallel over
heads, data-parallel over batch).  Each core produces a partial [T, D]
output (its heads' contribution through the matching wo rows); the host
sums the 4 partials per batch element.

Numerics: bf16 matmul operands throughout.  Scores are computed transposed
(sT[k, q] = k . q) so that P@V needs no transpose and softmax needs no max
subtraction (RMS-normed q/k bound |s| <= sqrt(128)); per-query exp-sums come
from a ones-column matmul accumulated in PSUM; normalization applied after
attention via a partition-broadcast multiply.

v2: diagonal blocks of the causal attention are sliced to the valid query
range (15% less score/exp/PV work); RMS statistics are computed with fused
tensor_tensor_reduce directly from PSUM; rope tables are bf16; output
partials are bf16; SBUF/PSUM live in single always-open pools with shared
PSUM tags so a repetition's phase-1 DMAs/matmuls overlap the previous
repetition's attention tail.
"""

import math

import numpy as np

D = 2048
HD = 128
NH = 16
NKV = 4
NQH = 4  # q heads per core
EPS = 1e-6
ROPE_THETA = 10000.0
N_CORES = 8

_dt = None
_nc_cache = {}


def _imports():
    global _dt, bass, mybir, tile, bacc, run_bass_kernel_spmd, make_identity, ExitStack
    import concourse.bass as bass
    import concourse.mybir as mybir
    import concourse.tile as tile
    from concourse import bacc
    from concourse.bass_utils import run_bass_kernel_spmd
    from concourse.masks import make_identity
    from contextlib import ExitStack
    _dt = mybir.dt


def build_nc(T=2048, reps=1):
    """Build the single-core Bass program (SPMD across 8 cores)."""
    _imports()
    dt = _dt
    f32 = dt.float32
    bf16 = dt.bfloat16
    TT = T // 128    # token tiles
    DC = D // 128    # contraction chunks
    QC = T // 512    # query chunks for attention
    SCALE = 1.0 / math.sqrt(HD)

    nc = bacc.Bacc()

    xTt = nc.dram_tensor("xTt", [TT, 128, D], bf16, kind="ExternalInput")
    wqT = nc.dram_tensor("wqT", [128, DC * NQH * HD], bf16, kind="ExternalInput")
    wkvT = nc.dram_tensor("wkvT", [128, DC * 2 * HD], bf16, kind="ExternalInput")
    woT = nc.dram_tensor("woT", [128, NQH * D], bf16, kind="ExternalInput")
    cosq = nc.dram_tensor("cosq", [128, T], bf16, kind="ExternalInput")
    sinqs = nc.dram_tensor("sinqs", [128, T], bf16, kind="ExternalInput")
    cosk = nc.dram_tensor("cosk", [128, T], bf16, kind="ExternalInput")
    sinks = nc.dram_tensor("sinks", [128, T], bf16, kind="ExternalInput")
    masks = nc.dram_tensor("masks", [128, 4 * 512], bf16, kind="ExternalInput")
    out = nc.dram_tensor("out", [T, D], bf16, kind="ExternalOutput")

    with nc.allow_low_precision(reason="bf16 matmul operands"), \
         tile.TileContext(nc) as tc, ExitStack() as octx:
        if reps > 1:
            octx.enter_context(tc.For_i(0, reps, 1))
        ctx = octx.enter_context(ExitStack())
        # Single SBUF pool for the whole body: phase-1 and phase-2 SBUF
        # tiles never alias, so repetition i+1's input DMAs can overlap
        # repetition i's attention/out-projection tail.  PSUM is too small
        # to share across phases, so it uses one pool per phase.
        sb = ctx.enter_context(tc.tile_pool(name="sb", bufs=1))

        ident = sb.tile([128, 128], bf16)
        make_identity(nc, ident[:])
        ones_col = sb.tile([128, 1], bf16)
        nc.vector.memset(ones_col[:], 1.0)
        eps_t = sb.tile([128, 1], f32)
        nc.vector.memset(eps_t[:], EPS)
        masks_sb = sb.tile([128, 4 * 512], bf16)
        nc.sync.dma_start(masks_sb[:], masks[:, :])

        # Persistent per-head transposed activations: [HD, T] each.
        qT_sb = sb.tile([128, NQH * T], bf16)
        kT_sb = sb.tile([128, T], bf16)
        v_sb = sb.tile([128, T], bf16)
        attT_sb = sb.tile([128, NQH * T], bf16)

        # ---------------- Phase 1: QKV projection + RMSNorm + RoPE ----------
        # Consolidated loads: one DMA per logical tensor.  The first
        # x tile and the weights are emitted first so the opening
        # matmuls' dependencies are at the front of the DMA queues.
        x0 = sb.tile([128, DC * 128], bf16, tag="x", bufs=3, name="x0")
        nc.sync.dma_start(x0[:], xTt[0, :, :])
        wq_sb = sb.tile([128, DC * NQH * HD], bf16, tag="wq")
        half = DC * NQH * HD // 2
        nc.sync.dma_start(wq_sb[:, :half], wqT[:, :half])
        nc.sync.dma_start(wq_sb[:, half:], wqT[:, half:])
        wkv_sb = sb.tile([128, DC * 2 * HD], bf16, tag="wkv")
        nc.sync.dma_start(wkv_sb[:], wkvT[:, :])
        cq_sb = sb.tile([128, T], bf16, tag="cq")
        nc.sync.dma_start(cq_sb[:], cosq[:, :])
        sq_sb = sb.tile([128, T], bf16, tag="sq")
        nc.sync.dma_start(sq_sb[:], sinqs[:, :])
        ck_sb = sb.tile([128, T], bf16, tag="ck")
        nc.sync.dma_start(ck_sb[:], cosk[:, :])
        sk_sb = sb.tile([128, T], bf16, tag="sk")
        nc.sync.dma_start(sk_sb[:], sinks[:, :])
        woT_sb = sb.tile([128, NQH * D], bf16, tag="woT")
        nc.sync.dma_start(woT_sb[:], woT[:, :])

        def rope_norm(tile_in, cos_t, sin_t, nh, tt, r_col, out_t,
                      scale_on_act):
            """tile_in [128, nh*128] (may be PSUM) -> rope'd+scaled out_t."""
            w = nh * HD
            m1 = sb.tile([128, 512], f32, tag="m1", name="m1")[:, :w]
            m2 = sb.tile([128, 512], f32, tag="m2", name="m2")[:, :w]
            base = tile_in
            # m1 = q * cos (cos broadcast across heads)
            cosv = bass.AP(cos_t.tensor, cos_t.offset + tt * 128,
                           [list(cos_t.ap[0])[:2], [0, nh], [1, HD]])
            nc.vector.tensor_mul(
                m1.rearrange("p (h c) -> p h c", h=nh), base.rearrange(
                    "p (h c) -> p h c", h=nh), cosv)
            # m2 = rot(q) * sin_signed
            rotv = bass.AP(base.tensor, base.offset + 64,
                           [list(base.ap[0])[:2], [HD, nh], [-64, 2], [1, 64]])
            sinv = bass.AP(sin_t.tensor, sin_t.offset + tt * 128,
                           [list(sin_t.ap[0])[:2], [0, nh], [64, 2], [1, 64]])
            nc.vector.tensor_mul(
                m2.rearrange("p (h r c) -> p h r c", h=nh, r=2, c=64),
                rotv, sinv)
            nc.vector.tensor_add(m1, m1, m2)
            # per-head rms scale (ACT for q side, DVE for k side)
            for h in range(nh):
                if scale_on_act:
                    nc.scalar.mul(
                        out_t[:, h * HD:(h + 1) * HD],
                        m1[:, h * HD:(h + 1) * HD], r_col[:, h:h + 1])
                else:
                    nc.vector.tensor_scalar_mul(
                        out_t[:, h * HD:(h + 1) * HD],
                        m1[:, h * HD:(h + 1) * HD], r_col[:, h:h + 1])

        with tc.tile_pool(name="p1ps", bufs=2, space="PSUM") as p1ps:

            def emit_transposes(tt, qf, kf):
                """transpose q (4 heads) and k of tile tt into [HD, T]."""
                for h in range(NQH):
                    tp = p1ps.tile([128, 128], bf16, tag="tp", name="tp")
                    nc.tensor.transpose(
                        tp[:], qf[:, h * HD:(h + 1) * HD], ident[:])
                    if h % 2 == 0:
                        nc.scalar.copy(
                            qT_sb[:, h * T + tt * 128:
                                     h * T + (tt + 1) * 128], tp[:])
                    else:
                        nc.vector.tensor_copy(
                            qT_sb[:, h * T + tt * 128:
                                     h * T + (tt + 1) * 128], tp[:])
                tp = p1ps.tile([128, 128], bf16, tag="tp", name="tp")
                nc.tensor.transpose(tp[:], kf[:], ident[:])
                nc.vector.tensor_copy(
                    kT_sb[:, tt * 128:(tt + 1) * 128], tp[:])

            pending = None
            for tt in range(TT):
                if tt == 0:
                    x_t = x0
                else:
                    x_t = sb.tile([128, DC * 128], bf16, tag="x", bufs=3,
                                  name=f"x{tt}")
                    nc.sync.dma_start(x_t[:], xTt[tt, :, :])
                q_ps = p1ps.tile([128, 512], f32, tag="qps", name="qps")
                for dc in range(DC):
                    nc.tensor.matmul(
                        q_ps[:], x_t[:, dc * 128:(dc + 1) * 128],
                        wq_sb[:, dc * 512:(dc + 1) * 512],
                        start=(dc == 0), stop=(dc == DC - 1))
                kv_ps = p1ps.tile([128, 256], f32, tag="kvps", name="kvps")
                for dc in range(DC):
                    nc.tensor.matmul(
                        kv_ps[:], x_t[:, dc * 128:(dc + 1) * 128],
                        wkv_sb[:, dc * 256:(dc + 1) * 256],
                        start=(dc == 0), stop=(dc == DC - 1))
                # previous tile's transposes go here so PE never waits on
                # the current tile's rope/norm chain
                if pending is not None:
                    emit_transposes(*pending)

                # v: straight copy of kv_ps[:, 128:]
                nc.vector.tensor_copy(
                    v_sb[:, tt * 128:(tt + 1) * 128], kv_ps[:, HD:2 * HD])

                # squares + copies (ACT), then free-axis reduce (DVE)
                q_sb2 = sb.tile([128, 512], f32, tag="q", bufs=2)
                nc.scalar.copy(q_sb2[:], q_ps[:])
                sqq = sb.tile([128, 512], f32, tag="sqq", bufs=2)
                nc.scalar.square(sqq[:], q_ps[:])
                k_sb2 = sb.tile([128, 128], f32, tag="k", bufs=2)
                nc.scalar.copy(k_sb2[:], kv_ps[:, 0:HD])
                sqk = sb.tile([128, 128], f32, tag="sqk", bufs=2)
                nc.scalar.square(sqk[:], kv_ps[:, 0:HD])
                ss = sb.tile([128, 8], f32, tag="ss", bufs=2)
                nc.vector.reduce_sum(
                    ss.rearrange("p (h one) -> p h one", h=8)[:, 0:4, :],
                    sqq.rearrange("p (h c) -> p h c", h=4),
                    axis=mybir.AxisListType.X)
                nc.vector.reduce_sum(
                    ss.rearrange("p (h one) -> p h one", h=8)[:, 4:5, :],
                    sqk.rearrange("p (h c) -> p h c", h=1),
                    axis=mybir.AxisListType.X)
                rt = sb.tile([128, 8], f32, tag="rt", bufs=2)
                nc.scalar.activation(
                    rt[:, 0:5], ss[:, 0:5],
                    mybir.ActivationFunctionType.Sqrt,
                    scale=1.0 / HD, bias=eps_t[:])
                rc = sb.tile([128, 8], f32, tag="rc", bufs=2)
                nc.vector.reciprocal(rc[:, 0:5], rt[:, 0:5])

                qf = sb.tile([128, 512], bf16, tag="qf", bufs=3)
                rope_norm(q_sb2[:], cq_sb[:], sq_sb[:], NQH, tt,
                          rc[:, 0:4], qf, scale_on_act=True)
                kf = sb.tile([128, 128], bf16, tag="kf", bufs=3)
                rope_norm(k_sb2[:], ck_sb[:], sk_sb[:], 1, tt,
                          rc[:, 4:5], kf, scale_on_act=False)
                pending = (tt, qf, kf)
            emit_transposes(*pending)

        # ---------------- Phase 2+3: attention + out-projection -------------
        # Heads run in interleaved PAIRS (two independent chains per engine
        # queue), key tiles in pairs sharing one PSUM score tile so each
        # softmax exp is a single wide ACT instruction (the ~352-cycle
        # fixed ACTIVATE overhead amortizes), and PV/sum matmuls for a
        # pair-step are emitted one step late so PE never waits on the
        # current exp.  PSUM: sT 2x4K + attT 2x2K + (sums|ops) 2x2K.
        with tc.tile_pool(name="p2ps", bufs=2, space="PSUM") as p2ps:
            for qc in range(QC):
                nkt = 4 * qc + 4
                nps = nkt // 2
                for hp in range(NQH // 2):
                    heads = (2 * hp, 2 * hp + 1)
                    attT_ps = {}
                    sums_ps = {}
                    for h in heads:
                        attT_ps[h] = p2ps.tile([128, 512], f32, tag="attT",
                                               name=f"attps{h}")
                        sums_ps[h] = p2ps.tile([1, 512], f32, tag="BD",
                                               name=f"sums{h}")

                    def emit_pvs(ps_idx, exps, W, ws, _qc=qc):
                        kt0 = 2 * ps_idx
                        for h in heads:
                            for j, kt in enumerate((kt0, kt0 + 1)):
                                nc.tensor.matmul(
                                    attT_ps[h][:, ws:],
                                    v_sb[:, kt * 128:(kt + 1) * 128],
                                    exps[h][:, j * W:(j + 1) * W],
                                    start=(kt == 0), stop=(kt == nkt - 1))
                                nc.tensor.matmul(
                                    sums_ps[h][:, ws:], ones_col[:],
                                    exps[h][:, j * W:(j + 1) * W],
                                    start=(kt == 0), stop=(kt == nkt - 1))

                    prev = None
                    for ps_i in range(nps):
                        kt0 = 2 * ps_i
                        ws = 128 * max(0, kt0 - 4 * qc)
                        W = 512 - ws
                        # scores for both heads: two key tiles packed into
                        # one PSUM tile
                        sTs = {}
                        for h in heads:
                            t_ = p2ps.tile([128, 1024], f32, tag="sT",
                                           name=f"sT{h}_{qc}_{ps_i}")
                            q_rhs = bass.AP(
                                qT_sb.tensor,
                                qT_sb.offset + h * T + qc * 512 + ws,
                                [list(qT_sb.ap[0])[:2], [1, W]])
                            nc.tensor.matmul(
                                t_[:, :W],
                                kT_sb[:, kt0 * 128:(kt0 + 1) * 128],
                                q_rhs, start=True, stop=True)
                            nc.tensor.matmul(
                                t_[:, W:2 * W],
                                kT_sb[:, (kt0 + 1) * 128:(kt0 + 2) * 128],
                                q_rhs, start=True, stop=True)
                            sTs[h] = t_
                        # one wide exp per head; window masks on diagonal
                        exps = {}
                        for h in heads:
                            expT = sb.tile([128, 1024], bf16, tag="expT",
                                           bufs=6)
                            nc.scalar.activation(
                                expT[:, :2 * W], sTs[h][:, :2 * W],
                                mybir.ActivationFunctionType.Exp,
                                scale=SCALE)
                            if kt0 >= 4 * qc:
                                j0 = kt0 - 4 * qc
                                nc.vector.tensor_mul(
                                    expT[:, :W], expT[:, :W],
                                    masks_sb[:, j0 * 512 + ws:
                                                j0 * 512 + 512])
                                nc.vector.tensor_mul(
                                    expT[:, W:2 * W], expT[:, W:2 * W],
                                    masks_sb[:, (j0 + 1) * 512 + ws:
                                                (j0 + 1) * 512 + 512])
                            exps[h] = expT
                        if prev is not None:
                            emit_pvs(*prev)
                        prev = (ps_i, exps, W, ws)
                    emit_pvs(*prev)
                    for h in heads:
                        recip = sb.tile([1, 512], f32, tag="recip", bufs=2)
                        nc.vector.reciprocal(recip[:], sums_ps[h][:])
                        rbc_sb = sb.tile([128, 512], f32, tag="rbcsb",
                                         bufs=2)
                        nc.gpsimd.partition_broadcast(rbc_sb[:], recip[:])
                        nc.vector.tensor_mul(
                            attT_sb[:, h * T + qc * 512:
                                       h * T + (qc + 1) * 512],
                            attT_ps[h][:], rbc_sb[:])

                # out-projection for the 4 token tiles of this q-chunk
                for tt in range(4 * qc, 4 * qc + 4):
                    o_sb = sb.tile([128, D], bf16, tag="osb", bufs=4)
                    for ns in range(D // 512):
                        o_ps = p2ps.tile([128, 512], f32, tag="BD",
                                         name="ops")
                        for h in range(NQH):
                            nc.tensor.matmul(
                                o_ps[:],
                                attT_sb[:, h * T + tt * 128:
                                           h * T + (tt + 1) * 128],
                                woT_sb[:, h * D + ns * 512:
                                          h * D + (ns + 1) * 512],
                                start=(h == 0), stop=(h == NQH - 1))
                        nc.vector.tensor_copy(
                            o_sb[:, ns * 512:(ns + 1) * 512], o_ps[:])
                    nc.sync.dma_start(
                        out[tt * 128:(tt + 1) * 128, :], o_sb[:])

    nc.compile()
    return nc


def _rope_tables(T, w):
    """cos/sin tables with norm weight folded; sin pre-rotated + signed."""
    inv_freq = 1.0 / (ROPE_THETA ** (np.arange(0, HD, 2, dtype=np.float32) / HD))
    t = np.arange(T, dtype=np.float32)
    ang = np.concatenate([np.outer(t, inv_freq)] * 2, axis=1)  # [T, HD]
    cos = np.cos(ang).astype(np.float32)
    sin = np.sin(ang).astype(np.float32)
    w = w.astype(np.float32)
    cosw = cos * w[None, :]
    sinw = np.concatenate(
        [-sin[:, :64] * w[None, 64:], sin[:, 64:] * w[None, :64]], axis=1)
    return np.ascontiguousarray(cosw), np.ascontiguousarray(sinw)


def _ttile(a, T):
    """[T, W] -> [128, TT*W] with column block tt holding rows tt*128.."""
    TT = T // 128
    W = a.shape[1]
    return np.ascontiguousarray(
        a.reshape(TT, 128, W).transpose(1, 0, 2).reshape(128, TT * W))


def _prep_core(x, wq, wk, wv, wo, q_norm_w, k_norm_w, b, g, T):
    TT, DC = T // 128, D // 128
    xb = np.ascontiguousarray(x[b], dtype=np.float32)
    # xTt[tt, p, dc*128+tp] = xb[tt*128+tp, dc*128+p]
    xTt = np.ascontiguousarray(
        xb.reshape(TT, 128, DC, 128).transpose(0, 3, 2, 1).reshape(
            TT, 128, DC * 128))
    wq_g = wq[512 * g:512 * (g + 1)]
    wqT = _chunked_T(wq_g, DC)          # [128, DC*512]
    kv = np.concatenate([wk[HD * g:HD * (g + 1)], wv[HD * g:HD * (g + 1)]], 0)
    wkvT = _chunked_T(kv, DC)           # [128, DC*256]
    wo_gT = np.ascontiguousarray(wo[:, 512 * g:512 * (g + 1)].T)  # [512, D]
    woT = np.ascontiguousarray(
        wo_gT.reshape(NQH, 128, D).transpose(1, 0, 2).reshape(128, NQH * D))
    cosq, sinqs = _rope_tables(T, q_norm_w)
    cosk, sinks = _rope_tables(T, k_norm_w)
    k_idx = np.arange(128)[:, None]
    q_idx = np.arange(512)[None, :]
    masks = np.concatenate(
        [(i * 128 + k_idx <= q_idx).astype(np.float32) for i in range(4)],
        axis=1)                          # [128, 4*512]
    import ml_dtypes
    bf = ml_dtypes.bfloat16
    return {
        "xTt": xTt.astype(bf), "wqT": wqT.astype(bf),
        "wkvT": wkvT.astype(bf), "woT": woT.astype(bf),
        "cosq": _ttile(cosq, T).astype(bf), "sinqs": _ttile(sinqs, T).astype(bf),
        "cosk": _ttile(cosk, T).astype(bf), "sinks": _ttile(sinks, T).astype(bf),
        "masks": np.ascontiguousarray(masks).astype(bf),
    }


def _chunked_T(w, DC):
    """[M, D] weights -> [128, DC*M]: chunk dc at cols dc*M, rows = d within chunk."""
    M = w.shape[0]
    wT = np.ascontiguousarray(w.T)      # [D, M]
    return np.ascontiguousarray(
        wT.reshape(DC, 128, M).transpose(1, 0, 2).reshape(128, DC * M))


LAST_EXEC_TIME_NS = None


def kernel(x, wq, wk, wv, wo, q_norm_w, k_norm_w):
    global LAST_EXEC_TIME_NS
    _imports()
    from concourse.bass_utils import run_bass_kernel_spmd

    T = x.shape[1]
    if T not in _nc_cache:
        _nc_cache[T] = build_nc(T)
    nc = _nc_cache[T]

    in_maps = []
    for c in range(N_CORES):
        b, g = c % 2, c // 2
        in_maps.append(_prep_core(np.asarray(x, dtype=np.float32),
                                  np.asarray(wq, dtype=np.float32),
                                  np.asarray(wk, dtype=np.float32),
                                  np.asarray(wv, dtype=np.float32),
                                  np.asarray(wo, dtype=np.float32),
                                  np.asarray(q_norm_w, dtype=np.float32),
                                  np.asarray(k_norm_w, dtype=np.float32),
                                  b, g, T))

    res = run_bass_kernel_spmd(nc, in_maps, core_ids=list(range(N_CORES)))
    LAST_EXEC_TIME_NS = res.exec_time_ns

    B = x.shape[0]
    out = np.zeros((B, T, D), dtype=np.float32)
    for c in range(N_CORES):
        b, g = c % 2, c // 2
        out[b] += res.results[c]["out"].astype(np.float32)
    return out


# revision 18
# speedup vs baseline: 2.3194x; 2.3194x over previous
"""Trainium2 Bass kernel for nn_Attention_49323404427915.

GQA attention block (B=2, T=2048, D=2048, 16 q-heads, 4 kv-heads, hd=128)
with per-head QK RMSNorm + RoPE + causal SDPA + out-projection.

Sharding over 8 cores: core c handles batch (c % 2) and q-head group
(c // 2) of 4 consecutive q-heads sharing one kv head (tensor-parallel over
heads, data-parallel over batch).  Each core produces a partial [T, D]
output (its heads' contribution through the matching wo rows); the host
sums the 4 partials per batch element.

Numerics: bf16 matmul operands throughout.  Scores are computed transposed
(sT[k, q] = k . q) so that P@V needs no transpose and softmax needs no max
subtraction (RMS-normed q/k bound |s| <= sqrt(128)); per-query exp-sums come
from a ones-column matmul accumulated in PSUM; normalization applied after
attention via a partition-broadcast multiply.

v2: diagonal blocks of the causal attention are sliced to the valid query
range (15% less score/exp/PV work); RMS statistics are computed with fused
tensor_tensor_reduce directly from PSUM; rope tables are bf16; output
partials are bf16; SBUF/PSUM live in single always-open pools with shared
PSUM tags so a repetition's phase-1 DMAs/matmuls overlap the previous
repetition's attention tail.
"""

import math

import numpy as np

D = 2048
HD = 128
NH = 16
NKV = 4
NQH = 4  # q heads per core
EPS = 1e-6
ROPE_THETA = 10000.0
N_CORES = 8

_dt = None
_nc_cache = {}


def _imports():
    global _dt, bass, mybir, tile, bacc, run_bass_kernel_spmd, make_identity, ExitStack
    import concourse.bass as bass
    import concourse.mybir as mybir
    import concourse.tile as tile
    from concourse import bacc
    from concourse.bass_utils import run_bass_kernel_spmd
    from concourse.masks import make_identity
    from contextlib import ExitStack
    _dt = mybir.dt


def build_nc(T=2048, reps=1):
    """Build the single-core Bass program (SPMD across 8 cores)."""
    _imports()
    dt = _dt
    f32 = dt.float32
    bf16 = dt.bfloat16
    TT = T // 128    # token tiles
    DC = D // 128    # contraction chunks
    QC = T // 512    # query chunks for attention
    SCALE = 1.0 / math.sqrt(HD)

    nc = bacc.Bacc()

    xTt = nc.dram_tensor("xTt", [TT, 128, D], bf16, kind="ExternalInput")
    wqT = nc.dram_tensor("wqT", [128, DC * NQH * HD], bf16, kind="ExternalInput")
    wkvT = nc.dram_tensor("wkvT", [128, DC * 2 * HD], bf16, kind="ExternalInput")
    woT = nc.dram_tensor("woT", [128, NQH * D], bf16, kind="ExternalInput")
    cosq = nc.dram_tensor("cosq", [128, T], bf16, kind="ExternalInput")
    sinqs = nc.dram_tensor("sinqs", [128, T], bf16, kind="ExternalInput")
    cosk = nc.dram_tensor("cosk", [128, T], bf16, kind="ExternalInput")
    sinks = nc.dram_tensor("sinks", [128, T], bf16, kind="ExternalInput")
    tri = nc.dram_tensor("tri", [128, 128], bf16, kind="ExternalInput")
    out = nc.dram_tensor("out", [T, D], bf16, kind="ExternalOutput")

    with nc.allow_low_precision(reason="bf16 matmul operands"), \
         tile.TileContext(nc) as tc, ExitStack() as octx:
        if reps > 1:
            octx.enter_context(tc.For_i(0, reps, 1))
        ctx = octx.enter_context(ExitStack())
        # Single SBUF pool for the whole body: phase-1 and phase-2 SBUF
        # tiles never alias, so repetition i+1's input DMAs can overlap
        # repetition i's attention/out-projection tail.  PSUM is too small
        # to share across phases, so it uses one pool per phase.
        sb = ctx.enter_context(tc.tile_pool(name="sb", bufs=1))

        ident = sb.tile([128, 128], bf16)
        make_identity(nc, ident[:])
        ones_col = sb.tile([128, 1], bf16)
        nc.vector.memset(ones_col[:], 1.0)
        eps_t = sb.tile([128, 1], f32)
        nc.vector.memset(eps_t[:], EPS)
        tri_sb = sb.tile([128, 128], bf16)
        nc.sync.dma_start(tri_sb[:], tri[:, :])

        # Persistent per-head transposed activations: [HD, T] each.
        qT_sb = sb.tile([128, NQH * T], bf16)
        kT_sb = sb.tile([128, T], bf16)
        v_sb = sb.tile([128, T], bf16)
        attT_sb = sb.tile([128, NQH * T], bf16)

        # ---------------- Phase 1: QKV projection + RMSNorm + RoPE ----------
        # Consolidated loads: one DMA per logical tensor.  The first
        # x tile and the weights are emitted first so the opening
        # matmuls' dependencies are at the front of the DMA queues.
        x0 = sb.tile([128, DC * 128], bf16, tag="x", bufs=3, name="x0")
        nc.sync.dma_start(x0[:], xTt[0, :, :])
        wq_sb = sb.tile([128, DC * NQH * HD], bf16, tag="wq")
        half = DC * NQH * HD // 2
        nc.sync.dma_start(wq_sb[:, :half], wqT[:, :half])
        nc.sync.dma_start(wq_sb[:, half:], wqT[:, half:])
        wkv_sb = sb.tile([128, DC * 2 * HD], bf16, tag="wkv")
        nc.sync.dma_start(wkv_sb[:], wkvT[:, :])
        cq_sb = sb.tile([128, T], bf16, tag="cq")
        nc.sync.dma_start(cq_sb[:], cosq[:, :])
        sq_sb = sb.tile([128, T], bf16, tag="sq")
        nc.sync.dma_start(sq_sb[:], sinqs[:, :])
        ck_sb = sb.tile([128, T], bf16, tag="ck")
        nc.sync.dma_start(ck_sb[:], cosk[:, :])
        sk_sb = sb.tile([128, T], bf16, tag="sk")
        nc.sync.dma_start(sk_sb[:], sinks[:, :])
        woT_sb = sb.tile([128, NQH * D], bf16, tag="woT")
        nc.sync.dma_start(woT_sb[:], woT[:, :])

        def rope_norm(tile_in, cos_t, sin_t, nh, tt, r_col, out_t,
                      scale_on_act):
            """tile_in [128, nh*128] (may be PSUM) -> rope'd+scaled out_t."""
            w = nh * HD
            m1 = sb.tile([128, 512], f32, tag="m1", name="m1")[:, :w]
            m2 = sb.tile([128, 512], f32, tag="m2", name="m2")[:, :w]
            base = tile_in
            # m1 = q * cos (cos broadcast across heads)
            cosv = bass.AP(cos_t.tensor, cos_t.offset + tt * 128,
                           [list(cos_t.ap[0])[:2], [0, nh], [1, HD]])
            nc.vector.tensor_mul(
                m1.rearrange("p (h c) -> p h c", h=nh), base.rearrange(
                    "p (h c) -> p h c", h=nh), cosv)
            # m2 = rot(q) * sin_signed
            rotv = bass.AP(base.tensor, base.offset + 64,
                           [list(base.ap[0])[:2], [HD, nh], [-64, 2], [1, 64]])
            sinv = bass.AP(sin_t.tensor, sin_t.offset + tt * 128,
                           [list(sin_t.ap[0])[:2], [0, nh], [64, 2], [1, 64]])
            nc.vector.tensor_mul(
                m2.rearrange("p (h r c) -> p h r c", h=nh, r=2, c=64),
                rotv, sinv)
            nc.vector.tensor_add(m1, m1, m2)
            # per-head rms scale (ACT for q side, DVE for k side)
            for h in range(nh):
                if scale_on_act:
                    nc.scalar.mul(
                        out_t[:, h * HD:(h + 1) * HD],
                        m1[:, h * HD:(h + 1) * HD], r_col[:, h:h + 1])
                else:
                    nc.vector.tensor_scalar_mul(
                        out_t[:, h * HD:(h + 1) * HD],
                        m1[:, h * HD:(h + 1) * HD], r_col[:, h:h + 1])

        with tc.tile_pool(name="p1ps", bufs=2, space="PSUM") as p1ps:

            def emit_transposes(tt, qf, kf):
                """transpose q (4 heads) and k of tile tt into [HD, T]."""
                for h in range(NQH):
                    tp = p1ps.tile([128, 128], bf16, tag="tp", name="tp")
                    nc.tensor.transpose(
                        tp[:], qf[:, h * HD:(h + 1) * HD], ident[:])
                    if h % 2 == 0:
                        nc.scalar.copy(
                            qT_sb[:, h * T + tt * 128:
                                     h * T + (tt + 1) * 128], tp[:])
                    else:
                        nc.vector.tensor_copy(
                            qT_sb[:, h * T + tt * 128:
                                     h * T + (tt + 1) * 128], tp[:])
                tp = p1ps.tile([128, 128], bf16, tag="tp", name="tp")
                nc.tensor.transpose(tp[:], kf[:], ident[:])
                nc.vector.tensor_copy(
                    kT_sb[:, tt * 128:(tt + 1) * 128], tp[:])

            pending = None
            for tt in range(TT):
                if tt == 0:
                    x_t = x0
                else:
                    x_t = sb.tile([128, DC * 128], bf16, tag="x", bufs=3,
                                  name=f"x{tt}")
                    nc.sync.dma_start(x_t[:], xTt[tt, :, :])
                q_ps = p1ps.tile([128, 512], f32, tag="qps", name="qps")
                for dc in range(DC):
                    nc.tensor.matmul(
                        q_ps[:], x_t[:, dc * 128:(dc + 1) * 128],
                        wq_sb[:, dc * 512:(dc + 1) * 512],
                        start=(dc == 0), stop=(dc == DC - 1))
                kv_ps = p1ps.tile([128, 256], f32, tag="kvps", name="kvps")
                for dc in range(DC):
                    nc.tensor.matmul(
                        kv_ps[:], x_t[:, dc * 128:(dc + 1) * 128],
                        wkv_sb[:, dc * 256:(dc + 1) * 256],
                        start=(dc == 0), stop=(dc == DC - 1))
                # previous tile's transposes go here so PE never waits on
                # the current tile's rope/norm chain
                if pending is not None:
                    emit_transposes(*pending)

                # v: straight copy of kv_ps[:, 128:]
                nc.vector.tensor_copy(
                    v_sb[:, tt * 128:(tt + 1) * 128], kv_ps[:, HD:2 * HD])

                # squares + copies (ACT), then free-axis reduce (DVE)
                q_sb2 = sb.tile([128, 512], f32, tag="q", bufs=2)
                nc.scalar.copy(q_sb2[:], q_ps[:])
                sqq = sb.tile([128, 512], f32, tag="sqq", bufs=2)
                nc.scalar.square(sqq[:], q_ps[:])
                k_sb2 = sb.tile([128, 128], f32, tag="k", bufs=2)
                nc.scalar.copy(k_sb2[:], kv_ps[:, 0:HD])
                sqk = sb.tile([128, 128], f32, tag="sqk", bufs=2)
                nc.scalar.square(sqk[:], kv_ps[:, 0:HD])
                ss = sb.tile([128, 8], f32, tag="ss", bufs=2)
                nc.vector.reduce_sum(
                    ss.rearrange("p (h one) -> p h one", h=8)[:, 0:4, :],
                    sqq.rearrange("p (h c) -> p h c", h=4),
                    axis=mybir.AxisListType.X)
                nc.vector.reduce_sum(
                    ss.rearrange("p (h one) -> p h one", h=8)[:, 4:5, :],
                    sqk.rearrange("p (h c) -> p h c", h=1),
                    axis=mybir.AxisListType.X)
                rt = sb.tile([128, 8], f32, tag="rt", bufs=2)
                nc.scalar.activation(
                    rt[:, 0:5], ss[:, 0:5],
                    mybir.ActivationFunctionType.Sqrt,
                    scale=1.0 / HD, bias=eps_t[:])
                rc = sb.tile([128, 8], f32, tag="rc", bufs=2)
                nc.vector.reciprocal(rc[:, 0:5], rt[:, 0:5])

                qf = sb.tile([128, 512], bf16, tag="qf", bufs=3)
                rope_norm(q_sb2[:], cq_sb[:], sq_sb[:], NQH, tt,
                          rc[:, 0:4], qf, scale_on_act=True)
                kf = sb.tile([128, 128], bf16, tag="kf", bufs=3)
                rope_norm(k_sb2[:], ck_sb[:], sk_sb[:], 1, tt,
                          rc[:, 4:5], kf, scale_on_act=False)
                pending = (tt, qf, kf)
            emit_transposes(*pending)

        # ---------------- Phase 2+3: attention + out-projection -------------
        # Heads are processed in interleaved PAIRS: two independent
        # score->exp->PV chains live in every engine queue at once, hiding
        # cross-engine semaphore latency that a single serial chain exposes.
        # PSUM (16 KiB): sT 4x2K + attT 2x2K + (sums|ops shared) 2x2K.
        with tc.tile_pool(name="p2ps", bufs=2, space="PSUM") as p2ps:
            for qc in range(QC):
                nkt = 4 * qc + 4
                for pair in range(NQH // 2):
                    heads = (2 * pair, 2 * pair + 1)
                    attT_ps = {}
                    sums_ps = {}
                    for h in heads:
                        attT_ps[h] = p2ps.tile([128, 512], f32, tag="attT",
                                               name=f"attps{h}")
                        sums_ps[h] = p2ps.tile([1, 512], f32, tag="BD",
                                               name=f"sums{h}")
                    sT_tiles = {}

                    def emit_sT(h, kt, _qc=qc):
                        # diagonal tiles only cover queries >= first key
                        i = max(0, kt - 4 * _qc)
                        W = 512 - 128 * i
                        t_ = p2ps.tile([128, 512], f32, tag="sT", bufs=4,
                                       name=f"sT{h}_{_qc}_{kt}")
                        q_rhs = bass.AP(
                            qT_sb.tensor,
                            qT_sb.offset + h * T + _qc * 512 + 128 * i,
                            [list(qT_sb.ap[0])[:2], [1, W]])
                        nc.tensor.matmul(
                            t_[:, :W], kT_sb[:, kt * 128:(kt + 1) * 128],
                            q_rhs, start=True, stop=True)
                        sT_tiles[(h, kt)] = t_

                    for h in heads:
                        emit_sT(h, 0)
                    for kt in range(nkt):
                        if kt + 1 < nkt:
                            for h in heads:
                                emit_sT(h, kt + 1)
                        i = max(0, kt - 4 * qc)
                        W = 512 - 128 * i
                        for h in heads:
                            sT_ps = sT_tiles.pop((h, kt))
                            expT = sb.tile([128, 512], bf16, tag="expT",
                                           bufs=6)
                            nc.scalar.activation(
                                expT[:, :W], sT_ps[:, :W],
                                mybir.ActivationFunctionType.Exp,
                                scale=SCALE)
                            if kt >= 4 * qc:
                                # triangular boundary block: first 128 cols
                                nc.vector.tensor_mul(
                                    expT[:, :128], expT[:, :128], tri_sb[:])
                            nc.tensor.matmul(
                                attT_ps[h][:, 128 * i:],
                                v_sb[:, kt * 128:(kt + 1) * 128],
                                expT[:, :W],
                                start=(kt == 0), stop=(kt == nkt - 1))
                            nc.tensor.matmul(
                                sums_ps[h][:, 128 * i:], ones_col[:],
                                expT[:, :W],
                                start=(kt == 0), stop=(kt == nkt - 1))
                    for h in heads:
                        recip = sb.tile([1, 512], f32, tag="recip", bufs=2)
                        nc.vector.reciprocal(recip[:], sums_ps[h][:])
                        rbc_sb = sb.tile([128, 512], f32, tag="rbcsb",
                                         bufs=2)
                        nc.gpsimd.partition_broadcast(rbc_sb[:], recip[:])
                        nc.vector.tensor_mul(
                            attT_sb[:, h * T + qc * 512:
                                       h * T + (qc + 1) * 512],
                            attT_ps[h][:], rbc_sb[:])

                # out-projection for the 4 token tiles of this q-chunk
                for tt in range(4 * qc, 4 * qc + 4):
                    o_sb = sb.tile([128, D], bf16, tag="osb", bufs=4)
                    for ns in range(D // 512):
                        o_ps = p2ps.tile([128, 512], f32, tag="BD",
                                         name="ops")
                        for h in range(NQH):
                            nc.tensor.matmul(
                                o_ps[:],
                                attT_sb[:, h * T + tt * 128:
                                           h * T + (tt + 1) * 128],
                                woT_sb[:, h * D + ns * 512:
                                          h * D + (ns + 1) * 512],
                                start=(h == 0), stop=(h == NQH - 1))
                        nc.vector.tensor_copy(
                            o_sb[:, ns * 512:(ns + 1) * 512], o_ps[:])
                    nc.sync.dma_start(
                        out[tt * 128:(tt + 1) * 128, :], o_sb[:])

    nc.compile()
    return nc


def _rope_tables(T, w):
    """cos/sin tables with norm weight folded; sin pre-rotated + signed."""
    inv_freq = 1.0 / (ROPE_THETA ** (np.arange(0, HD, 2, dtype=np.float32) / HD))
    t = np.arange(T, dtype=np.float32)
    ang = np.concatenate([np.outer(t, inv_freq)] * 2, axis=1)  # [T, HD]
    cos = np.cos(ang).astype(np.float32)
    sin = np.sin(ang).astype(np.float32)
    w = w.astype(np.float32)
    cosw = cos * w[None, :]
    sinw = np.concatenate(
        [-sin[:, :64] * w[None, 64:], sin[:, 64:] * w[None, :64]], axis=1)
    return np.ascontiguousarray(cosw), np.ascontiguousarray(sinw)


def _ttile(a, T):
    """[T, W] -> [128, TT*W] with column block tt holding rows tt*128.."""
    TT = T // 128
    W = a.shape[1]
    return np.ascontiguousarray(
        a.reshape(TT, 128, W).transpose(1, 0, 2).reshape(128, TT * W))


def _prep_core(x, wq, wk, wv, wo, q_norm_w, k_norm_w, b, g, T):
    TT, DC = T // 128, D // 128
    xb = np.ascontiguousarray(x[b], dtype=np.float32)
    # xTt[tt, p, dc*128+tp] = xb[tt*128+tp, dc*128+p]
    xTt = np.ascontiguousarray(
        xb.reshape(TT, 128, DC, 128).transpose(0, 3, 2, 1).reshape(
            TT, 128, DC * 128))
    wq_g = wq[512 * g:512 * (g + 1)]
    wqT = _chunked_T(wq_g, DC)          # [128, DC*512]
    kv = np.concatenate([wk[HD * g:HD * (g + 1)], wv[HD * g:HD * (g + 1)]], 0)
    wkvT = _chunked_T(kv, DC)           # [128, DC*256]
    wo_gT = np.ascontiguousarray(wo[:, 512 * g:512 * (g + 1)].T)  # [512, D]
    woT = np.ascontiguousarray(
        wo_gT.reshape(NQH, 128, D).transpose(1, 0, 2).reshape(128, NQH * D))
    cosq, sinqs = _rope_tables(T, q_norm_w)
    cosk, sinks = _rope_tables(T, k_norm_w)
    k_idx = np.arange(128)[:, None]
    q_idx = np.arange(128)[None, :]
    tri = (k_idx <= q_idx).astype(np.float32)     # [128, 128]
    import ml_dtypes
    bf = ml_dtypes.bfloat16
    return {
        "xTt": xTt.astype(bf), "wqT": wqT.astype(bf),
        "wkvT": wkvT.astype(bf), "woT": woT.astype(bf),
        "cosq": _ttile(cosq, T).astype(bf), "sinqs": _ttile(sinqs, T).astype(bf),
        "cosk": _ttile(cosk, T).astype(bf), "sinks": _ttile(sinks, T).astype(bf),
        "tri": np.ascontiguousarray(tri).astype(bf),
    }


def _chunked_T(w, DC):
    """[M, D] weights -> [128, DC*M]: chunk dc at cols dc*M, rows = d within chunk."""
    M = w.shape[0]
    wT = np.ascontiguousarray(w.T)      # [D, M]
    return np.ascontiguousarray(
        wT.reshape(DC, 128, M).transpose(1, 0, 2).reshape(128, DC * M))


LAST_EXEC_TIME_NS = None


def kernel(x, wq, wk, wv, wo, q_norm_w, k_norm_w):
    global LAST_EXEC_TIME_NS
    _imports()
    from concourse.bass_utils import run_bass_kernel_spmd

    T = x.shape[1]
    if T not in _nc_cache:
        _nc_cache[T] = build_nc(T)
    nc = _nc_cache[T]

    in_maps = []
    for c in range(N_CORES):
        b, g = c % 2, c // 2
        in_maps.append(_prep_core(np.asarray(x, dtype=np.float32),
                                  np.asarray(wq, dtype=np.float32),
                                  np.asarray(wk, dtype=np.float32),
                                  np.asarray(wv, dtype=np.float32),
                                  np.asarray(wo, dtype=np.float32),
                                  np.asarray(q_norm_w, dtype=np.float32),
                                  np.asarray(k_norm_w, dtype=np.float32),
                                  b, g, T))

    res = run_bass_kernel_spmd(nc, in_maps, core_ids=list(range(N_CORES)))
    LAST_EXEC_TIME_NS = res.exec_time_ns

    B = x.shape[0]
    out = np.zeros((B, T, D), dtype=np.float32)
    for c in range(N_CORES):
        b, g = c % 2, c // 2
        out[b] += res.results[c]["out"].astype(np.float32)
    return out
